# revision 2
# baseline (speedup 1.0000x reference)
"""DILATE loss (soft-DTW shape + temporal) on 8 Trainium2 NeuronCores.

Strategy (data-parallel, per the sharding hint): the 256 independent
(batch x channel) series are sharded 32 per core; each core runs its own
128x128 DP per series with series on SBUF partitions; the scalar loss is
reduced on the host.

Per-core algorithm (gamma=0.01 makes softmin ultra-sharp, so a min-plus
DP with a pseudo-posterior gradient matches the reference closely):
  D[i,j]   = (t_i - o_j)^2
  M[i,j]   = D[i,j] + min(M[i-1,j-1], M[i-1,j], M[i,j-1])        (forward Viterbi)
  num[i,j] = D[i,j] + min(num[i,j+1], num[i+1,j], num[i+1,j+1])  (suffix Viterbi)
  E*Omega  = exp(-lam*(M - D + num - M[N,N] + womg)),  womg = -ln(Omega)/lam
  vals     = M[N,N];   tl = sum_ij (E*Omega)[i,j]
  loss     = 0.5*sum(vals)/B + 0.5*sum(tl)/(B*T*T)

Each DP row is one TT-min + one tensor_tensor_scan (min,add) on the DVE;
the D build / suffix-term / exp / reduce phases are bulk ops overlapped
across GPSIMD / ACT / DVE by the Tile scheduler.
"""
import sys
if "/opt/trn_rl_repo" not in sys.path:
    sys.path.insert(0, "/opt/trn_rl_repo")
import numpy as np
from contextlib import ExitStack

import concourse.bass as bass
import concourse.bacc as bacc
import concourse.mybir as mybir
import concourse.tile as tile
from concourse.mybir import AluOpType, ActivationFunctionType

F32 = mybir.dt.float32
S = 32          # series per core
N = 128         # DP size (= T)
LAM = 100.0     # 1/gamma
BIG = 1e30
RS = N + 1      # row stride in the stores (value cols + 1 guard/boundary col)
N_CORES = 8


def _ap(t, off, dims):
    base = t[:]
    return bass.AP(base.tensor, base.offset + off, [base.ap[0]] + dims)


def _build_kernel():
    nc = bacc.Bacc("TRN2", target_bir_lowering=False, debug=False)
    t_d = nc.dram_tensor("t", [S, N], F32, kind="ExternalInput")
    o_d = nc.dram_tensor("o", [S, N], F32, kind="ExternalInput")
    omg_d = nc.dram_tensor("omg", [S, N * N], F32, kind="ExternalInput")
    vals_d = nc.dram_tensor("vals", [S, 1], F32, kind="ExternalOutput")
    tl_d = nc.dram_tensor("tl", [S, 1], F32, kind="ExternalOutput")

    with tile.TileContext(nc) as tc, ExitStack() as ctx:
        pool = ctx.enter_context(tc.tile_pool(name="main", bufs=1))
        t_s = pool.tile([S, N], F32, tag="t_s")
        o_s = pool.tile([S, N], F32, tag="o_s")
        omg_s = pool.tile([S, N * N], F32, tag="omg_s")
        D_s = pool.tile([S, RS * N], F32, tag="D_s")
        M_s = pool.tile([S, RS * (N + 1)], F32, tag="M_s")
        ent_s = pool.tile([S, N], F32, tag="ent_s")
        vals_s = pool.tile([S, 1], F32, tag="vals_s")
        bias_s = pool.tile([S, 1], F32, tag="bias_s")
        tl_s = pool.tile([S, 1], F32, tag="tl_s")

        nc.sync.dma_start(t_s[:], t_d.ap())
        nc.sync.dma_start(o_s[:], o_d.ap())
        nc.sync.dma_start(omg_s[:], omg_d.ap())

        # guards/boundaries
        nc.gpsimd.memset(_ap(D_s, N, [[RS, N], [1, 1]]), BIG)
        nc.gpsimd.memset(_ap(M_s, 0, [[RS, N + 1], [1, 1]]), BIG)
        nc.gpsimd.memset(_ap(M_s, 1, [[1, N]]), BIG)
        nc.gpsimd.memset(_ap(M_s, 0, [[1, 1]]), 0.0)

        # D build, chunked: gpsimd diff -> ACT square (overlaps forward rows)
        d_rows = _ap(D_s, 0, [[RS, N], [1, N]])
        CH = 16
        for c0 in range(0, N, CH):
            dch = _ap(D_s, c0 * RS, [[RS, CH], [1, N]])
            t_ch = _ap(t_s, c0, [[1, CH], [0, N]])
            o_ch = _ap(o_s, 0, [[0, CH], [1, N]])
            nc.gpsimd.tensor_tensor(dch, t_ch, o_ch, AluOpType.subtract)
            nc.scalar.activation(dch, dch, ActivationFunctionType.Square)

        # forward min-plus rows
        for r in range(1, N + 1):
            prev = (r - 1) * RS
            nc.vector.tensor_tensor(
                ent_s[:], _ap(M_s, prev, [[1, N]]), _ap(M_s, prev + 1, [[1, N]]),
                AluOpType.min)
            nc.vector.tensor_tensor_scan(
                _ap(M_s, r * RS + 1, [[1, N]]), ent_s[:],
                _ap(D_s, (r - 1) * RS, [[1, N]]),
                BIG, AluOpType.min, AluOpType.add)

        # y bulk on gpsimd (overlaps forward): omg := womg - D
        om_rows = _ap(omg_s, 0, [[N, N], [1, N]])
        nc.gpsimd.tensor_tensor(om_rows, om_rows, d_rows, AluOpType.subtract)

        # vals = M[N,N]; bias = +lam*M[N,N]
        nc.vector.tensor_copy(vals_s[:], _ap(M_s, N * RS + N, [[1, 1]]))
        nc.vector.tensor_scalar(bias_s[:], vals_s[:], LAM, None, AluOpType.mult)

        # backward (suffix) min-plus rows; num overwrites D rows in place
        entb_s = ent_s
        for r in range(N, 0, -1):
            if r == N:
                nc.gpsimd.memset(entb_s[:], BIG)
                nc.gpsimd.memset(entb_s[:, N - 1:N], 0.0)
            else:
                nc.vector.tensor_tensor(
                    entb_s[:], _ap(D_s, r * RS, [[1, N]]), _ap(D_s, r * RS + 1, [[1, N]]),
                    AluOpType.min)
            drow_rev = _ap(D_s, (r - 1) * RS + N - 1, [[-1, N]])
            nc.vector.tensor_tensor_scan(
                drow_rev, _ap(entb_s, N - 1, [[-1, N]]), drow_rev,
                BIG, AluOpType.min, AluOpType.add)

        # E*Omega, chunked: arg = M + num + (womg - D); EOm = exp(-lam*arg + lam*MNN)
        tlp_s = pool.tile([S, N // CH], F32, tag="tlp_s")
        for ci, c0 in enumerate(range(0, N, CH)):
            mch = _ap(M_s, (c0 + 1) * RS + 1, [[RS, CH], [1, N]])
            dch = _ap(D_s, c0 * RS, [[RS, CH], [1, N]])
            och = _ap(omg_s, c0 * N, [[N, CH], [1, N]])
            nc.vector.tensor_tensor(mch, mch, dch, AluOpType.add)
            nc.vector.tensor_tensor(mch, mch, och, AluOpType.add)
            nc.scalar.activation(och, mch, ActivationFunctionType.Exp,
                                 bias=bias_s[:], scale=-LAM)
            nc.vector.tensor_reduce(tlp_s[:, ci:ci + 1],
                                    _ap(omg_s, c0 * N, [[1, CH * N]]),
                                    mybir.AxisListType.X, AluOpType.add)
        nc.vector.tensor_reduce(tl_s[:], tlp_s[:], mybir.AxisListType.X, AluOpType.add)

        nc.sync.dma_start(vals_d.ap(), vals_s[:])
        nc.sync.dma_start(tl_d.ap(), tl_s[:])

    nc.compile()
    return nc


_NC_CACHE = None
_OMG_CACHE = None


def _get_nc():
    global _NC_CACHE
    if _NC_CACHE is None:
        _NC_CACHE = _build_kernel()
    return _NC_CACHE


def _womg():
    global _OMG_CACHE
    if _OMG_CACHE is None:
        idx = np.arange(1, N + 1, dtype=np.float64)
        om2d = ((idx[:, None] - idx[None, :]) ** 2).reshape(N * N)
        w = np.where(om2d == 0.0, BIG, -np.log(np.maximum(om2d, 1e-30)) / LAM)
        _OMG_CACHE = np.ascontiguousarray(
            np.broadcast_to(w.astype(np.float32), (S, N * N)))
    return _OMG_CACHE


_EXEC_CACHE = None


def _get_exec():
    """Build the sharded jitted executable once (mirrors bass2jax's
    run_bass_via_pjrt multi-core path) and keep the big constant omg input
    resident on the devices."""
    global _EXEC_CACHE
    if _EXEC_CACHE is not None:
        return _EXEC_CACHE
    import jax
    import concourse.mybir as _mybir
    from jax.sharding import Mesh, PartitionSpec, NamedSharding
    from jax.experimental.shard_map import shard_map
    from concourse.bass2jax import (
        _bass_exec_p, install_neuronx_cc_hook, partition_id_tensor)

    nc = _get_nc()
    install_neuronx_cc_hook()
    partition_name = nc.partition_id_tensor.name if nc.partition_id_tensor else None
    in_names, out_names, out_avals, zero_outs = [], [], [], []
    for alloc in nc.m.functions[0].allocations:
        if not isinstance(alloc, _mybir.MemoryLocationSet):
            continue
        name = alloc.memorylocations[0].name
        if alloc.kind == "ExternalInput":
            if name != partition_name:
                in_names.append(name)
        elif alloc.kind == "ExternalOutput":
            shape = tuple(alloc.tensor_shape)
            dtype = _mybir.dt.np(alloc.dtype)
            out_names.append(name)
            out_avals.append(jax.core.ShapedArray(shape, dtype))
            zero_outs.append(np.zeros(shape, dtype))
    n_params = len(in_names)
    all_in_names = list(in_names) + list(out_names)
    if partition_name is not None:
        all_in_names.append(partition_name)
    donate = tuple(range(n_params, n_params + len(out_names)))

    def _body(*args):
        operands = list(args)
        if partition_name is not None:
            operands.append(partition_id_tensor())
        return tuple(_bass_exec_p.bind(
            *operands,
            out_avals=tuple(out_avals),
            in_names=tuple(all_in_names),
            out_names=tuple(out_names),
            lowering_input_output_aliases=(),
            sim_require_finite=True,
            sim_require_nnan=True,
            nc=nc,
        ))

    devices = jax.devices()[:N_CORES]
    mesh = Mesh(np.asarray(devices), ("core",))
    in_specs = (PartitionSpec("core"),) * (n_params + len(out_names))
    out_specs = (PartitionSpec("core"),) * len(out_names)
    sharded = jax.jit(
        shard_map(_body, mesh=mesh, in_specs=in_specs, out_specs=out_specs,
                  check_rep=False),
        donate_argnums=donate, keep_unused=True)
    shard = NamedSharding(mesh, PartitionSpec("core"))
    omg_dev = jax.device_put(
        np.concatenate([_womg()] * N_CORES, axis=0), shard)
    _EXEC_CACHE = (sharded, in_names, out_names, zero_outs, shard, omg_dev)
    return _EXEC_CACHE


def kernel(outputs, targets):
    """outputs, targets: [64, 128, 4] float32 -> scalar float32 loss."""
    sharded, in_names, out_names, zero_outs, shard, omg_dev = _get_exec()
    outputs = np.asarray(outputs, np.float32)
    targets = np.asarray(targets, np.float32)
    B, T, C = outputs.shape
    t = np.ascontiguousarray(np.transpose(targets, (0, 2, 1)).reshape(B * C, T))
    o = np.ascontiguousarray(np.transpose(outputs, (0, 2, 1)).reshape(B * C, T))
    by_name = {"t": t, "o": o, "omg": omg_dev}
    concat_in = [by_name[name] for name in in_names]
    concat_zeros = [
        np.zeros((N_CORES * z.shape[0], *z.shape[1:]), z.dtype) for z in zero_outs
    ]
    out_arrs = sharded(*concat_in, *concat_zeros)
    outs = {name: np.asarray(out_arrs[i]) for i, name in enumerate(out_names)}
    vals = outs["vals"][:, 0]
    tl = outs["tl"][:, 0]
    loss = 0.5 * (vals.sum(dtype=np.float64) / B) + \
           0.5 * (tl.sum(dtype=np.float64) / (B * T * T))
    return np.float32(loss)
